# revision 34
# baseline (speedup 1.0000x reference)
"""CausalBiTrilinearBCNAttention Trainium2 kernel (feature-major, scan-based).

Math refactorization: every use of Q, K, invQ, invK in the
reference is through a rank-R projection, and causal cumsum commutes with
right multiplication, so the network collapses to

    xp  = x @ P                  P = [a1|a2s|a3|b1|b2|b3|b7]  (D x 448)
    cum = causal_cumsum(xp[..., 192:448]) / counts
    g1  = xp[:,0:64]*cum[b1] + xp[:,64:128]*cum[b2]
    g2  = xp[:,128:192]*cum[b3]*cum[b7]
    out = [g1|g2] @ A.T          A = [WO@U_b | alpha_tri*WO@U_t]  (D x 128)

The dataflow is FEATURE-major ([feat partitions, token free]) end to end,
streamed in two token-half phases so the post-stream work of half 0
overlaps the phase-2 matmuls:

  - xp.T = P_chunk.T @ xT_chunk accumulated over 8 D-chunks into 4 PSUM
    group tiles per token-half ([a1|a2s] 128p, [a3] 64p, [b1|b2] 128p,
    [b3|b7] 128p, x 512 tokens = 8 banks total).
  - causal cumsum of the RAW b-groups on the Vector engine via
    tensor_tensor_scan along the token (free) axis, fp32 state, chained
    per 256-token quarter; the T/2 carry for second-half cores enters as
    the scan `initial` (host sends sxPb = sum(x_first_half) @ P_b).
  - 1/counts normalization is algebraically hoisted: with raw cumsums c
    and counts n,  g1 = (a1*c1 + a2*c2)/n  and  g2 = ((a3*c3)*(c7/n))/n,
    so only c7 and the two assembled G halves get an invcb multiply
    (invcb is built on PE by a ones-matmul broadcast of the host
    1/counts row during the initial DMA window, which also pre-warms
    the HAM clock gate).
  - elementwise products read the A-group PSUMs directly (mixed
    PSUM/SBUF operands dodge the equal-base-partition rule); products
    on Vector, adds/second products on GpSimd.
  - finals: AT[128g,128d] stationary @ G[128g,256t] per (quarter,
    d-chunk-pair) into ping-ponged PSUM slots reclaimed from the
    stream tags, fp16 copies split Vector/Scalar, one batched output
    DMA per (quarter, 4-chunk half) on the sync queue.

Sharding: 8 cores = 4 batches x 2 T-halves, as v1.
"""

import numpy as np

import concourse.bass as bass
import concourse.tile as tile
from concourse import bacc, mybir
from concourse.bass_utils import run_bass_kernel_spmd
from concourse.alu_op_type import AluOpType

B, T, D, R = 4, 2048, 1024, 64
TH = T // 2          # tokens per core
ND = D // 128        # 8 d chunks
PCOLS = 448          # 7 * R
NQ = 4               # token quarters in the tail pipeline
QT = TH // NQ        # 256

F32 = mybir.dt.float32
F16 = mybir.dt.float16


def build_nc():
    nc = bacc.Bacc(None, target_bir_lowering=False)

    xT = nc.dram_tensor("xT", [D, TH], F16, kind="ExternalInput")
    P = nc.dram_tensor("P", [D, PCOLS], F16, kind="ExternalInput")
    AT = nc.dram_tensor("AT", [128, D], F16, kind="ExternalInput")
    invcr = nc.dram_tensor("invcr", [1, TH], F16, kind="ExternalInput")
    sxPb = nc.dram_tensor("sxPb", [128, 2], F32, kind="ExternalInput")
    outT = nc.dram_tensor("outT", [D, TH], F16, kind="ExternalOutput")

    with tile.TileContext(nc) as tc:
        with tc.tile_pool(name="consts", bufs=1) as consts, \
             tc.tile_pool(name="big", bufs=1) as big, \
             tc.tile_pool(name="ewp", bufs=2) as ewp, \
             tc.tile_pool(name="outp", bufs=2) as outp, \
             tc.tile_pool(name="ps", bufs=1, space="PSUM") as ps:

            # ---- SBUF ----
            xT_sb = big.tile([128, ND, TH], F16)
            P_sb = consts.tile([128, ND, PCOLS], F16)
            AT_sb = consts.tile([128, D], F16)
            invcr_sb = consts.tile([1, TH], F16)
            sxPb_sb = consts.tile([128, 2], F32)
            onesc_sb = consts.tile([1, 128], F16)
            warm_sb = consts.tile([128, 512], F16)

            invcb_sb = big.tile([128, TH], F16)
            sc0_sb = big.tile([128, TH], F16)    # raw cumsum of [b1|b2]
            sc1_sb = big.tile([128, TH], F16)    # raw cumsum of [b3|b7]
            c7n_sb = big.tile([64, TH], F16)     # c7/n at base 0
            gw_sb = [big.tile([128, 512], F16, name=f"gw{h}")
                     for h in range(2)]          # [g1raw | g2raw] pre-scale
            G_sb = [big.tile([128, 512], F16, name=f"G{h}")
                    for h in range(2)]

            # ---- PSUM: 4 groups x 2 token-half phases = 8 banks ----
            psA0 = [ps.tile([128, 512], F32, tag=f"A0h{h}", bufs=1,
                            name=f"psA0h{h}") for h in range(2)]
            psA1 = [ps.tile([64, 512], F32, tag=f"A1h{h}", bufs=1,
                            name=f"psA1h{h}") for h in range(2)]
            psB0 = [ps.tile([128, 512], F32, tag=f"B0h{h}", bufs=1,
                            name=f"psB0h{h}") for h in range(2)]
            psB1 = [ps.tile([128, 512], F32, tag=f"B1h{h}", bufs=1,
                            name=f"psB1h{h}") for h in range(2)]

            nc.gpsimd.memset(warm_sb, 0.0)
            nc.vector.memset(onesc_sb, 1.0)

            # ---- DMA issues: phase-1 (tokens 0:512) first, few big issues
            xTv = xT.rearrange("(k p) t -> p k t", p=128)
            Pv = P.rearrange("(k p) c -> p k c", p=128)
            outv = outT.rearrange("(k p) t -> p k t", p=128)
            HA = slice(0, 512)
            HB = slice(512, 1024)

            nc.sync.dma_start(out=P_sb[:, 0:1, :], in_=Pv[:, 0:1, :])
            nc.scalar.dma_start(out=invcr_sb, in_=invcr[:, :])
            nc.scalar.dma_start(out=sxPb_sb, in_=sxPb[:, :])
            nc.sync.dma_start(out=xT_sb[:, 0:1, HA], in_=xTv[:, 0:1, HA])
            nc.scalar.dma_start(out=P_sb[:, 1:2, :], in_=Pv[:, 1:2, :])
            nc.sync.dma_start(out=xT_sb[:, 1:2, HA], in_=xTv[:, 1:2, HA])
            nc.scalar.dma_start(out=P_sb[:, 2:5, :], in_=Pv[:, 2:5, :])
            nc.sync.dma_start(out=xT_sb[:, 2:4, HA], in_=xTv[:, 2:4, HA])
            nc.scalar.dma_start(out=P_sb[:, 5:8, :], in_=Pv[:, 5:8, :])
            nc.sync.dma_start(out=xT_sb[:, 4:6, HA], in_=xTv[:, 4:6, HA])
            nc.scalar.dma_start(out=xT_sb[:, 0:3, HB], in_=xTv[:, 0:3, HB])
            nc.sync.dma_start(out=xT_sb[:, 6:8, HA], in_=xTv[:, 6:8, HA])
            nc.scalar.dma_start(out=xT_sb[:, 3:6, HB], in_=xTv[:, 3:6, HB])
            nc.scalar.dma_start(out=xT_sb[:, 6:8, HB], in_=xTv[:, 6:8, HB])
            nc.sync.dma_start(out=AT_sb, in_=AT[:, :])

            # ---- PE warmup during the DMA latency window ----
            for i in range(5):
                nc.tensor.matmul(psA0[0], warm_sb[:, 0:128], warm_sb,
                                 start=True, stop=True)
            # invcb = broadcast of 1/counts row to 128 partitions (K=1 mm)
            for h, s in ((0, HA), (1, HB)):
                nc.tensor.matmul(psB0[h], onesc_sb, invcr_sb[:, s],
                                 start=True, stop=True)
                nc.vector.tensor_copy(invcb_sb[:, s], psB0[h])

            groups = [
                (psB0, 192, 320, 128),   # [b1|b2]
                (psB1, 320, 448, 128),   # [b3|b7]
                (psA0, 0, 128, 128),     # [a1|a2s]
                (psA1, 128, 192, 64),    # [a3]
            ]

            def stream_phase(h, s):
                # dk-pair blocks, group-major inside: consecutive matmuls
                # share a PSUM bank, halving bank/acc-group transitions.
                for j in range(ND // 2):
                    for (pst, c0, c1, m) in groups:
                        for dk in (2 * j, 2 * j + 1):
                            st, sp = (dk == 0), (dk == ND - 1)
                            nc.tensor.matmul(pst[h][0:m, :],
                                             P_sb[:, dk, c0:c1],
                                             xT_sb[:, dk, s],
                                             start=st, stop=sp)

            # Tail per 256-token quarter q (half h = q // 2, w = q % 2):
            #   both scans on Vector (from PSUM, quarter-chained via
            #   `initial`), c7n + adds on GpSimd, products on Vector,
            #   G assembled per half with a single invcb multiply.
            def tail_quarter(q):
                h, w = q // 2, q % 2
                s = slice(q * QT, (q + 1) * QT)       # sbuf token cols
                pw = slice(w * QT, (w + 1) * QT)      # psum token cols
                i0 = sxPb_sb[:, 0:1] if q == 0 else sc0_sb[:, q * QT - 1:q * QT]
                i1 = sxPb_sb[:, 1:2] if q == 0 else sc1_sb[:, q * QT - 1:q * QT]
                nc.vector.tensor_tensor_scan(sc0_sb[:, s], psB0[h][:, pw],
                                             invcb_sb[:, s], i0,
                                             AluOpType.add, AluOpType.bypass)
                nc.vector.tensor_tensor_scan(sc1_sb[:, s], psB1[h][:, pw],
                                             invcb_sb[:, s], i1,
                                             AluOpType.add, AluOpType.bypass)
                nc.gpsimd.tensor_mul(c7n_sb[:, s], sc1_sb[64:128, s],
                                     invcb_sb[64:128, s])
                # g1 = (a1*c1 + a2*c2)/n, g2 = ((a3*c3)*(c7/n))/n
                m2 = ewp.tile([64, QT], F16, tag="m2", bufs=4)
                nc.vector.tensor_mul(m2, psA0[h][64:128, pw],
                                     sc0_sb[64:128, s])
                g1r = ewp.tile([64, QT], F16, tag="g1r", bufs=4)
                nc.vector.tensor_mul(g1r, psA0[h][0:64, pw], sc0_sb[0:64, s])
                nc.gpsimd.tensor_add(gw_sb[h][0:64, pw], g1r, m2)
                u = ewp.tile([64, QT], F16, tag="u", bufs=4)
                nc.vector.tensor_mul(u, psA1[h][0:64, pw], sc1_sb[0:64, s])
                nc.gpsimd.tensor_mul(gw_sb[h][64:128, pw], u, c7n_sb[:, s])
                nc.vector.tensor_mul(G_sb[h][:, pw], gw_sb[h][:, pw],
                                     invcb_sb[:, s])

            def finals_quarter(q, act_only=False):
                h, w = q // 2, q % 2
                s = slice(q * QT, (q + 1) * QT)
                gsl = slice(w * QT, (w + 1) * QT)
                for half in range(2):
                    o_sb = outp.tile([128, 4, QT], F16, tag="osb", bufs=4)
                    for pp in range(2):
                        pair = half * 2 + pp
                        o_ps = ps.tile([128, 2, QT], F32,
                                       tag=["B0h0", "B1h0", "A0h0",
                                            "A1h0"][pair % 4], bufs=1,
                                       name=f"o_{q}_{pair}")
                        for j in range(2):
                            dk = pair * 2 + j
                            nc.tensor.matmul(
                                o_ps[:, j, :],
                                AT_sb[:, dk * 128:(dk + 1) * 128],
                                G_sb[h][:, gsl], start=True, stop=True)
                        dst = o_sb[:, pp * 2:pp * 2 + 2, :]
                        if pp == 0 and not act_only:
                            nc.vector.tensor_copy(dst, o_ps)
                        else:
                            nc.scalar.copy(dst, o_ps)
                    nc.sync.dma_start(out=outv[:, half * 4:half * 4 + 4, s],
                                      in_=o_sb)

            stream_phase(0, HA)
            tail_quarter(0)
            tail_quarter(1)
            stream_phase(1, HB)
            finals_quarter(0, act_only=True)
            finals_quarter(1, act_only=True)
            tail_quarter(2)
            tail_quarter(3)
            finals_quarter(2)
            finals_quarter(3)

    nc.finalize()
    return nc


_NC = None


def _get_nc():
    global _NC
    if _NC is None:
        _NC = build_nc()
    return _NC


def _fold_weights(WQ, WK, WO, Winv, U_b, V_b, W_b, U_t, V_t, W_t, X_t,
                  alpha_bi, alpha_tri):
    f8 = np.float64
    WQ, WK, WO, Winv = (np.asarray(m) for m in (WQ, WK, WO, Winv))
    U_b, V_b, W_b = (np.asarray(m) for m in (U_b, V_b, W_b))
    U_t, V_t, W_t, X_t = (np.asarray(m) for m in (U_t, V_t, W_t, X_t))
    WQt = WQ.astype(f8).T
    WKt = WK.astype(f8).T
    Winvt = Winv.astype(f8).T
    P = np.concatenate([
        WQt @ V_b.astype(f8),
        float(alpha_bi) * (WQt @ (Winvt @ W_b.astype(f8))),
        WQt @ V_t.astype(f8),
        WKt @ W_b.astype(f8),
        WKt @ (Winvt @ V_b.astype(f8)),
        WKt @ W_t.astype(f8),
        X_t.astype(f8),
    ], axis=1).astype(np.float32)
    A = np.concatenate([
        WO.astype(f8) @ U_b.astype(f8),
        float(alpha_tri) * (WO.astype(f8) @ U_t.astype(f8)),
    ], axis=1).astype(np.float32)
    return P, A


def make_in_maps(x, P, A):
    AT = np.ascontiguousarray(A.T.astype(np.float16))
    P16 = P.astype(np.float16)
    Pb = P16[:, 192:448].astype(np.float64)
    in_maps = []
    for core in range(8):
        b, h = core // 2, core % 2
        xTc = np.ascontiguousarray(x[b, h * TH:(h + 1) * TH, :].T
                                   .astype(np.float16))
        counts = np.arange(h * TH + 1, (h + 1) * TH + 1, dtype=np.float64)
        invcr = (1.0 / counts).astype(np.float16).reshape(1, TH)
        if h == 1:
            sx = x[b, :TH, :].astype(np.float16).astype(np.float64) \
                .sum(axis=0)
            sxP = (sx @ Pb).astype(np.float32)
        else:
            sxP = np.zeros(256, np.float32)
        sxPb = np.ascontiguousarray(sxP.reshape(2, 128).T)
        in_maps.append(dict(xT=xTc, P=P16, AT=AT, invcr=invcr, sxPb=sxPb))
    return in_maps


def kernel(x, WQ, WK, WO, Winv, U_b, V_b, W_b, bias_b,
           U_t, V_t, W_t, X_t, bias_t, alpha_bi, alpha_tri):
    x = np.asarray(x, dtype=np.float32)
    P, A = _fold_weights(WQ, WK, WO, Winv, U_b, V_b, W_b,
                         U_t, V_t, W_t, X_t, alpha_bi, alpha_tri)
    in_maps = make_in_maps(x, P, A)

    res = run_bass_kernel_spmd(_get_nc(), in_maps, core_ids=list(range(8)))

    out = np.empty((B, T, D), np.float32)
    for core in range(8):
        b, h = core // 2, core % 2
        out[b, h * TH:(h + 1) * TH, :] = \
            res.results[core]["outT"].T.astype(np.float32)

    # constant bias term (zero for the given inputs, kept for fidelity)
    bias_out = ((1.0 + float(alpha_bi)) * np.asarray(bias_b, np.float64)
                + float(alpha_tri) * np.asarray(bias_t, np.float64)) \
        @ np.asarray(WO, np.float64).T
    if np.any(bias_out):
        out += bias_out.astype(np.float32)[None, None, :]
    return out
